# revision 29
# baseline (speedup 1.0000x reference)
"""Multi-head attention (B=4, S=1024, D=1024, H=16) on 8 TRN2 NeuronCores.

Sharding: batch (4) x head-half (2) -> 8 cores, zero cross-core traffic.
Core c handles batch b = c // 2 and heads [hh*8, hh*8+8) where hh = c % 2.
Each core computes a partial output y_part[s, e] (its 512 channels fed
through its slice of Wo) in bf16; the host sums the two partials per batch
and adds the bias terms in f32.

On-device pipeline per core (matmul operands bf16, accumulation fp32):

Head (tracks input-DMA arrival; ~10us of runtime preamble before any DMA
data flows, then ~250GB/s/core shared across 8 cores):
  stretch 1 (d-outer, xv-paced):   V'[st0-3] partial = xv @ Wv'
  stretch 2 (d-outer, xq/xk-paced): V'[st4-7] + QT0 = Wq_0' @ xq (pre-scaled
    1/sqrt(dk)) + KT0 = Wk_0' @ xk — eight PSUM banks: 4 V halves + 2 Q + 2 K.

Cruise: 8 blocks = (head-pair j, q-chunk qn), j-major. Per block, per
k-tile round:
  ST   = KhT.T @ QhT           [k 128, q 512]  (K=64; both heads share one
                                2-bank PSUM tile -> one exp covers both)
  E    = exp(ST + mask_bias)   (ACT, fused mask — the pacing op: score
                                matmuls WAR-wait on exp via the 2-tile
                                PSUM rotation)
  AV of round kt-2 (lag 2, so its exp is ready):
  psO += Vaug.T @ E            [65, q 512] rows 0-63 = out_h^T, row 64 = denom
  + 2 filler matmuls: Q/K projections of pair j+1 (blocks 0-5) or the
    output projection rows 0-511 (block 7, 4/round).
  normalize at block end: 1/denom via native DVE Reciprocal, partition-
  broadcast on gpsimd (its only Q7 op type — mixing Q7 op types swaps
  libraries at ~7us each), multiply on DVE.

Tail: AV rounds 6-7 + normalize of block 7, then y rows 512-1023
(= concatT.T @ Wo'), stored as bf16 across 2 queues.
"""

import os
import sys

sys.path.insert(0, "/opt/trn_rl_repo")

import numpy as np
import ml_dtypes

BF16 = ml_dtypes.bfloat16

B, S, D = 4, 1024, 1024
HEADS = 16
DK = 64
P = 128
NCORES = 8
DCH = D // P       # 8 contraction chunks
PAIRS = 4          # head-pairs per core (8 heads / 2)
QN = 2             # q 512-chunks
KT = 8             # k tiles of 128
VW = 65            # V channels per head + ones column

_STATE = {}


def _build():
    """Build + compile the per-core Bass program (cached)."""
    if "nc" in _STATE:
        return _STATE["nc"]

    import concourse.bass as bass  # noqa: F401
    import concourse.mybir as mybir
    from concourse import bacc
    from concourse import tile

    f32 = mybir.dt.float32
    bf16 = mybir.dt.bfloat16
    AF = mybir.ActivationFunctionType
    ALU = mybir.AluOpType

    # Pin Exp/Ln to the one activation table containing both, so the
    # table-load pass never alternates tables between the softmax exp and
    # the ln/exp reciprocal (each ACT_TABLE_LOAD costs ~1.3us).
    _orig_tables = bacc.get_activation_tables

    def _pinned_tables(arch):
        t = dict(_orig_tables(arch))
        target = "natural_log_exp_and_others"
        if target in t:
            for k in t:
                if k != target:
                    t[k] = t[k] - {AF.Exp, AF.Ln}
        return t

    bacc.get_activation_tables = _pinned_tables

    nc = bacc.Bacc("TRN2", target_bir_lowering=False, debug=False)

    # All inputs staged host-side in partition-major layouts: each partition
    # row is 8-16KB contiguous, so DMA packets are large (the natural [D, S]
    # layouts yield 1-2KB packets and only ~110GB/s per ring).
    xq_d = nc.dram_tensor("xq", [P, DCH, S], bf16, kind="ExternalInput")
    xk_d = nc.dram_tensor("xk", [P, DCH, S], bf16, kind="ExternalInput")
    xv_d = nc.dram_tensor("xv", [P, DCH, S], bf16, kind="ExternalInput")
    wq_d = nc.dram_tensor("wq", [P, PAIRS, DCH, P], bf16, kind="ExternalInput")
    wk_d = nc.dram_tensor("wk", [P, PAIRS, DCH, P], bf16, kind="ExternalInput")
    wv_d = nc.dram_tensor("wv", [P, DCH, 512], bf16, kind="ExternalInput")
    wo_d = nc.dram_tensor("wo", [P, PAIRS, D], bf16, kind="ExternalInput")
    bq_d = nc.dram_tensor("bq", [P, PAIRS], f32, kind="ExternalInput")
    bk_d = nc.dram_tensor("bk", [P, PAIRS], f32, kind="ExternalInput")
    mb_d = nc.dram_tensor("mb", [P, KT], f32, kind="ExternalInput")
    y_d = nc.dram_tensor("y", [P, KT, D], bf16, kind="ExternalOutput")

    from contextlib import ExitStack

    with tile.TileContext(nc) as tc, ExitStack() as ctx:
        const = ctx.enter_context(tc.tile_pool(name="const", bufs=1))
        # Resident tensors
        wv_sb = const.tile([P, DCH, 512], bf16)
        wq_sb = const.tile([P, PAIRS, DCH, P], bf16)
        wk_sb = const.tile([P, PAIRS, DCH, P], bf16)
        wo_sb = const.tile([P, PAIRS, D], bf16)
        xq_sb = const.tile([P, DCH, S], bf16)
        xk_sb = const.tile([P, DCH, S], bf16)
        xv_sb = const.tile([P, DCH, S], bf16)
        v_sb = const.tile([P, KT, 8 * VW], bf16)
        cat_sb = const.tile([P, PAIRS, S], bf16)
        bq_sb = const.tile([P, PAIRS], f32)
        bk_sb = const.tile([P, PAIRS], f32)
        mb_sb = const.tile([P, KT], f32)

        # SBUF pools
        qtp = ctx.enter_context(tc.tile_pool(name="qtp", bufs=2))
        ktp = ctx.enter_context(tc.tile_pool(name="ktp", bufs=2))
        epool = ctx.enter_context(tc.tile_pool(name="epool", bufs=8))
        rpool = ctx.enter_context(tc.tile_pool(name="rpool", bufs=4))
        r2pool = ctx.enter_context(tc.tile_pool(name="r2pool", bufs=4))
        ypool = ctx.enter_context(tc.tile_pool(name="ypool", bufs=3))
        spool = ctx.enter_context(tc.tile_pool(name="spool", bufs=6))
        # PSUM: 8 banks. pssp 2x[128,2,512] = 4 banks (scores; V-half
        # accumulators in the head), psop 2x[128,512] = 2 banks (AV output;
        # Q-p0 in the head), psacc 2x[128,512] = 2 banks (projections, O).
        pssp = ctx.enter_context(tc.tile_pool(name="pssp", bufs=2, space="PSUM"))
        psop = ctx.enter_context(tc.tile_pool(name="psop", bufs=2, space="PSUM"))
        psacc = ctx.enter_context(tc.tile_pool(name="psacc", bufs=2, space="PSUM"))

        # ---------------- DMA issuance: one ring per x-tensor ----------------
        # Only sync/scalar/gpsimd can issue DMA; each x-tensor gets its own
        # ring, split into two half-transfers so the d-outer head loops can
        # start on the first half. Pair-0 weights lead their ring.
        nc.scalar.dma_start(wq_sb[:, 0], wq_d.ap()[:, 0])
        nc.sync.dma_start(wk_sb[:, 0], wk_d.ap()[:, 0])
        nc.gpsimd.dma_start(mb_sb[:], mb_d.ap())
        h = DCH // 2
        for half in range(2):
            sl = slice(half * h, (half + 1) * h)
            nc.scalar.dma_start(xq_sb[:, sl], xq_d.ap()[:, sl])
            nc.sync.dma_start(xk_sb[:, sl], xk_d.ap()[:, sl])
            nc.gpsimd.dma_start(wv_sb[:, sl], wv_d.ap()[:, sl])
        for half in range(2):
            sl = slice(half * h, (half + 1) * h)
            nc.gpsimd.dma_start(xv_sb[:, sl], xv_d.ap()[:, sl])
        nc.scalar.dma_start(bq_sb[:], bq_d.ap())
        nc.sync.dma_start(bk_sb[:], bk_d.ap())
        for j in range(1, PAIRS):
            nc.scalar.dma_start(wq_sb[:, j], wq_d.ap()[:, j])
            nc.sync.dma_start(wk_sb[:, j], wk_d.ap()[:, j])

        def evict_v(st, ps):
            vview = v_sb[:, st].rearrange("p (h c) -> p h c", c=VW)
            nc.vector.tensor_copy(
                vview[:, :, 0:64], ps.rearrange("p (h c) -> p h c", c=64)
            )

        # ---------------- Head stretch 1: V' st0-3, d-outer ----------------
        psva = [pssp.tile([P, 2, 512], f32, tag="s", name=f"psva{t}") for t in range(2)]
        for d in range(DCH):
            for st in range(4):
                nc.tensor.matmul(
                    psva[st // 2][:, st % 2],
                    xv_sb[:, d, st * P : (st + 1) * P],
                    wv_sb[:, d],
                    start=(d == 0),
                    stop=(d == DCH - 1),
                )
        for st in range(4):
            evict_v(st, psva[st // 2][:, st % 2])

        # ------- Head stretch 2: V' st4-7 + Q-p0 + K-p0, d-outer -------
        psvb = [pssp.tile([P, 2, 512], f32, tag="s", name=f"psvb{t}") for t in range(2)]
        psq0 = [psop.tile([P, 512], f32, tag="o", name=f"psq0_{n}") for n in range(QN)]
        psk0 = [psacc.tile([P, 512], f32, tag="acc", name=f"psk0_{n}") for n in range(QN)]
        for d in range(DCH):
            for st in range(4, 8):
                nc.tensor.matmul(
                    psvb[(st - 4) // 2][:, st % 2],
                    xv_sb[:, d, st * P : (st + 1) * P],
                    wv_sb[:, d],
                    start=(d == 0),
                    stop=(d == DCH - 1),
                )
            for n in range(QN):
                nc.tensor.matmul(
                    psq0[n],
                    wq_sb[:, 0, d],
                    xq_sb[:, d, n * 512 : (n + 1) * 512],
                    start=(d == 0),
                    stop=(d == DCH - 1),
                )
            for n in range(QN):
                nc.tensor.matmul(
                    psk0[n],
                    wk_sb[:, 0, d],
                    xk_sb[:, d, n * 512 : (n + 1) * 512],
                    start=(d == 0),
                    stop=(d == DCH - 1),
                )
        for st in range(4, 8):
            evict_v(st, psvb[(st - 4) // 2][:, st % 2])
        qts = [qtp.tile([P, S], bf16, tag="qt", name=f"qt{j}", bufs=4) for j in range(PAIRS)]
        kts = [ktp.tile([P, S], bf16, tag="kt", name=f"kt{j}") for j in range(PAIRS)]
        for n in range(QN):
            nc.vector.tensor_scalar_add(
                qts[0][:, n * 512 : (n + 1) * 512], psq0[n], bq_sb[:, 0:1]
            )
            nc.vector.tensor_scalar_add(
                kts[0][:, n * 512 : (n + 1) * 512], psk0[n], bk_sb[:, 0:1]
            )
        # ones columns for the denominator rows (bf16 memset can't stride here)
        ones_f32 = const.tile([P, KT, 8], f32)
        nc.vector.memset(ones_f32[:], 1.0)
        ones_view = v_sb.rearrange("p t (h c) -> p t h c", c=VW)[:, :, :, 64:65]
        nc.vector.tensor_copy(ones_view, ones_f32[:].unsqueeze(3))

        # wo is needed only by the output projection; issue it after the
        # priority streams so it never competes with xq/xk for HBM.
        nc.gpsimd.dma_start(wo_sb[:], wo_d.ap())

        # ---------------- Cruise: 8 software-pipelined blocks ----------------
        yq = [nc.sync, nc.gpsimd]

        def projqk_gen(j):
            """Yield the K then Q projection matmuls of pair j (one per next);
            each 8-matmul group's bias-add eviction follows its last yield.
            K first: the next pair's qn0 scores need all of kt but only the
            qn0 half of qt, so this order unblocks them earliest."""
            for wsb, xsb, bsb, out in (
                (wk_sb, xk_sb, bk_sb, kts[j]),
                (wq_sb, xq_sb, bq_sb, qts[j]),
            ):
                for n in range(QN):
                    ps = psacc.tile([P, 512], f32, tag="acc", name=f"pp{j}_{n}")
                    for d in range(DCH):
                        yield nc.tensor.matmul(
                            ps,
                            wsb[:, j, d],
                            xsb[:, d, n * 512 : (n + 1) * 512],
                            start=(d == 0),
                            stop=(d == DCH - 1),
                        )
                    nc.vector.tensor_scalar_add(
                        out[:, n * 512 : (n + 1) * 512], ps, bsb[:, j : j + 1]
                    )

        def o_gen(st_lo, st_hi):
            """Yield output-projection matmuls for s-row chunks [st_lo, st_hi)."""
            for st in range(st_lo, st_hi):
                for en in range(2):
                    psy = psacc.tile([P, 512], f32, tag="acc", name=f"psy{st}_{en}")
                    for cc in range(PAIRS):
                        yield nc.tensor.matmul(
                            psy,
                            cat_sb[:, cc, st * P : (st + 1) * P],
                            wo_sb[:, cc, en * 512 : (en + 1) * 512],
                            start=(cc == 0),
                            stop=(cc == PAIRS - 1),
                        )
                    ysb = ypool.tile([P, 512], bf16, tag="y", name=f"y{st}_{en}")
                    nc.vector.tensor_copy(ysb[:], psy)
                    yq[(st * 2 + en) % len(yq)].dma_start(
                        y_d.ap()[:, st, en * 512 : (en + 1) * 512], ysb[:]
                    )

        def av(b, j, ets, psos, kt):
            for sub in range(2):
                h = j * 2 + sub
                nc.tensor.matmul(
                    psos[sub],
                    v_sb[:, kt, h * VW : (h + 1) * VW],
                    ets[kt][:, sub],
                    start=(kt == 0),
                    stop=(kt == KT - 1),
                )

        def norm_front(b, psos):
            """Start normalizing block b: evict psO (freeing its PSUM bank
            fast — the DVE queue must not block here), 1/denom as
            exp(-ln(denom)) on ACT, partition-broadcast on gpsimd. Returns
            the deferred cat-multiply emitters."""
            j, qn = b // 2, b % 2
            stgs, r2s = [], []
            for sub in range(2):
                stg = spool.tile([VW, 512], f32, tag="stg", name=f"stg{b}_{sub}")
                nc.vector.tensor_copy(stg[:], psos[sub][:])
                stgs.append(stg)
            for sub in range(2):
                lrow = rpool.tile([1, 512], f32, tag="l", name=f"l{b}_{sub}")
                nc.scalar.activation(lrow[:], psos[sub][64:65, :], AF.Ln)
                rrow = rpool.tile([1, 512], f32, tag="r", name=f"r{b}_{sub}")
                nc.scalar.activation(rrow[:], lrow[:], AF.Exp, scale=-1.0)
                r2 = r2pool.tile([64, 512], f32, tag="r2", name=f"r2{b}_{sub}")
                nc.gpsimd.partition_broadcast(r2[:], rrow[:])
                r2s.append(r2)

            def mults():
                for sub in range(2):
                    lo = sub * 64
                    nc.vector.tensor_tensor(
                        cat_sb[lo : lo + 64, j, qn * 512 : (qn + 1) * 512],
                        stgs[sub][0:64, :],
                        r2s[sub][:],
                        op=ALU.mult,
                    )

            return mults

        # Dense-stretch emission (score matmuls consecutive — anything woven
        # between them delays the exp feed path and de-saturates ACT):
        # pair j: [cat-mults of older blocks] [proj j+1 | O rows 0-511]
        #         [scores qn0 | exps] [AV qn0] [scores qn1 | exps] [AV qn1]
        pending_mults = []

        def attn_block(b):
            j, qn = b // 2, b % 2
            ets = []
            psos = [
                psop.tile([VW, 512], f32, tag="o", name=f"pso{b}_{s}")
                for s in range(2)
            ]
            for kt in range(KT):
                pss = pssp.tile([P, 2, 512], f32, tag="s", name=f"pss{b}_{kt}")
                for sub in range(2):
                    lo, hi = sub * 64, (sub + 1) * 64
                    nc.tensor.matmul(
                        pss[:, sub],
                        kts[j][lo:hi, kt * P : (kt + 1) * P],
                        qts[j][lo:hi, qn * 512 : (qn + 1) * 512],
                        start=True,
                        stop=True,
                    )
                et = epool.tile([P, 2, 512], bf16, tag="e", name=f"e{b}_{kt}")
                nc.scalar.activation(
                    et[:], pss[:], AF.Exp, bias=mb_sb[:, kt : kt + 1], scale=1.0
                )
                ets.append(et)
            for kt in range(KT):
                av(b, j, ets, psos, kt)
            pending_mults.append(norm_front(b, psos))

        def flush_mults():
            while pending_mults:
                pending_mults.pop(0)()

        # Attention first in every pair — the exp stream (the pacing engine)
        # starts the moment the head's qt0/kt0 land; projections of the next
        # pair run in the exp stream's shadow after both attn blocks.
        for j in range(PAIRS):
            if j < 3:
                attn_block(2 * j)
                attn_block(2 * j + 1)
                flush_mults()
                for _ in projqk_gen(j + 1):
                    pass
            else:
                attn_block(6)
                flush_mults()  # includes block 6's cat, feeding O rows 0-511
                for _ in o_gen(0, 4):
                    pass
                attn_block(7)
                flush_mults()

        # Drain: y rows 512-1023. Open both psacc groups with their first
        # three steps so the PE works while the block-7 normalize (which
        # gates each group's last step, pair 3's qn1 cat) completes.
        for st in range(4, KT, 2):
            psys = []
            for st2 in (st, st + 1):
                for en in range(2):
                    psy = psacc.tile([P, 512], f32, tag="acc", name=f"psy{st2}_{en}")
                    psys.append((st2, en, psy))
            # psacc has 2 slots: run pairs of groups through cc0-2 then cc3
            for gi in range(0, 4, 2):
                for g in psys[gi : gi + 2]:
                    st2, en, psy = g
                    for cc in range(PAIRS - 1):
                        nc.tensor.matmul(
                            psy,
                            cat_sb[:, cc, st2 * P : (st2 + 1) * P],
                            wo_sb[:, cc, en * 512 : (en + 1) * 512],
                            start=(cc == 0),
                            stop=False,
                        )
                for st2, en, psy in psys[gi : gi + 2]:
                    nc.tensor.matmul(
                        psy,
                        cat_sb[:, PAIRS - 1, st2 * P : (st2 + 1) * P],
                        wo_sb[:, PAIRS - 1, en * 512 : (en + 1) * 512],
                        start=False,
                        stop=True,
                    )
                    ysb = ypool.tile([P, 512], bf16, tag="y", name=f"yd{st2}_{en}")
                    nc.vector.tensor_copy(ysb[:], psy)
                    yq[(st2 * 2 + en) % len(yq)].dma_start(
                        y_d.ap()[:, st2, en * 512 : (en + 1) * 512], ysb[:]
                    )

    nc.compile()
    _STATE["nc"] = nc
    return nc


def _pmajor_x(x):
    """[S?, D]-style [D, S] transpose folded to partition-major [P, DCH, S]."""
    # x: [S, D] activation; device wants xT chunked: out[p, d, s] = x[s, d*128+p]
    return np.ascontiguousarray(x.T.reshape(DCH, P, S).transpose(1, 0, 2)).astype(BF16)


def _shard(q, k, v, mask, Wq, bq, Wk, bk, Wv, bv, Wo, bo):
    """Build the 8 per-core input maps (host-side layout preparation).
    Everything is partition-major so DMA packets are large."""
    scale = 1.0 / np.sqrt(DK)
    in_maps = []
    for c in range(NCORES):
        b = c // 2
        hh = c % 2
        c0 = hh * 512
        wq_s = (Wq[c0 : c0 + 512, :] * scale).T  # [D, 512]
        wk_s = Wk[c0 : c0 + 512, :].T
        wv_s = Wv[c0 : c0 + 512, :].T
        wo_s = Wo[:, c0 : c0 + 512].T  # [512, D]
        mrow = mask[b, 0, 0, :]
        # wq_s [D, 512] -> [P, PAIRS, DCH, 128]: w[p,j,d,m] = wq_s[d*128+p, j*128+m]
        wq_p = wq_s.reshape(DCH, P, PAIRS, P).transpose(1, 2, 0, 3)
        wk_p = wk_s.reshape(DCH, P, PAIRS, P).transpose(1, 2, 0, 3)
        # wv_s [D, 512] -> [P, DCH, 512]
        wv_p = wv_s.reshape(DCH, P, 512).transpose(1, 0, 2)
        # wo_s [512, D] -> [P, PAIRS, D]: wo[p,cc,e] = wo_s[cc*128+p, e]
        wo_p = wo_s.reshape(PAIRS, P, D).transpose(1, 0, 2)
        in_maps.append(
            {
                "xq": _pmajor_x(q[b]),
                "xk": _pmajor_x(k[b]),
                "xv": _pmajor_x(v[b]),
                "wq": np.ascontiguousarray(wq_p).astype(BF16),
                "wk": np.ascontiguousarray(wk_p).astype(BF16),
                "wv": np.ascontiguousarray(wv_p).astype(BF16),
                "wo": np.ascontiguousarray(wo_p).astype(BF16),
                "bq": np.ascontiguousarray(
                    (bq[c0 : c0 + 512] * scale).reshape(PAIRS, P).T, dtype=np.float32
                ),
                "bk": np.ascontiguousarray(
                    bk[c0 : c0 + 512].reshape(PAIRS, P).T, dtype=np.float32
                ),
                "mb": np.ascontiguousarray(
                    np.where(mrow == 0, np.float32(-1e9), np.float32(0.0))
                    .astype(np.float32)
                    .reshape(KT, P)
                    .T
                ),
            }
        )
    return in_maps


def _gather(results, Wv, bv, Wo, bo):
    """Sum per-core bf16 partials into the full [B, S, D] f32 output."""
    # Channel-bias correction folded out of the device kernel: the V bias
    # passes through softmax-weighted sums with total weight 1, so its
    # contribution to y is the constant row Wo @ bv.
    corr = (Wo.astype(np.float64) @ bv.astype(np.float64)).astype(np.float32)
    y = np.empty((B, S, D), dtype=np.float32)
    for b in range(B):
        # device y is [P, KT, D]; s = st*128 + p
        ya = results[2 * b]["y"].astype(np.float32).transpose(1, 0, 2).reshape(S, D)
        yb = results[2 * b + 1]["y"].astype(np.float32).transpose(1, 0, 2).reshape(S, D)
        y[b] = ya + yb + corr + bo
    return y


def _run(trace=False, **inputs):
    import time

    from concourse.bass_utils import run_bass_kernel_spmd

    nc = _build()
    args = {k: np.asarray(v) for k, v in inputs.items()}
    in_maps = _shard(**args)
    last_err = None
    for attempt in range(3):
        try:
            res = run_bass_kernel_spmd(
                nc, in_maps, core_ids=list(range(NCORES)), trace=trace
            )
            break
        except Exception as e:  # device occasionally wedges; retry recovers
            last_err = e
            time.sleep(10 * (attempt + 1))
    else:
        raise last_err
    y = _gather(res.results, args["Wv"], args["bv"], args["Wo"], args["bo"])
    return y, res


def kernel(**inputs):
    y, _ = _run(trace=False, **inputs)
    return y
